# revision 21
# baseline (speedup 1.0000x reference)
"""DKVMN forward kernel for 8 Trainium2 NeuronCores (Bass/Tile) — v5.

The axon tunnel costs ~70-90 ms fixed latency PER sharded host->device
array plus ~55 MB/s bandwidth, and the dispatch itself has a ~90 ms
floor; actual device compute is only ~25 ms.  So v5 optimizes the
transfer, not the math:
 - ONE per-core 2-byte ExternalInput ("allin") = [blob shard | xw |
   kfidx16] (~354 KB/core).  All core-invariant data lives in a blob
   uploaded 1/8 per core and AllGathered on-device over NeuronLink.
 - k/v embedding tables stored fp8e4m3 (scaled x128; 1/128 folded into
   Mk/eaW/fW2 host-side), upconverted to bf16 on device.
 - Scan gather indices uploaded compact as xw[4, T*8] = 4*x; device
   replicates to 128 partitions (broadcast DMAs) and adds ks = p%4
   (int16 DVE add).  kf indices wrapped-in-16, replicated on device.
 - M0 stored compact [4, SEG] (rows depend only on p%4) and expanded
   on device with 32 DMAs; f32 scalars read via bitcast(F32) views.
 - pred output fp16.
Device side, the HW tensor_tensor_scan (~0.4 elem/cycle) is replaced
by an unrolled per-step recurrence on plain TT ops (~3.7 elem/cycle):
states buffer S[C+1] with carry in slot 0, batched D/U build per
chunk, one fused product+reduce pass for the reads.
"""
import sys
import numpy as np
import ml_dtypes

sys.path.insert(0, '/opt/trn_rl_repo')

import jax  # noqa: E402
for _k, _v in [("jax_compilation_cache_dir", "/tmp/jax_comp_cache"),
               ("jax_persistent_cache_min_compile_time_secs", 0.0),
               ("jax_persistent_cache_min_entry_size_bytes", 0)]:
    try:
        jax.config.update(_k, _v)
    except Exception:
        pass

import concourse.bass as bass          # noqa: E402
import concourse.bacc as bacc          # noqa: E402
import concourse.mybir as mybir        # noqa: E402
from concourse.tile import TileContext # noqa: E402
from concourse.bass_utils import run_bass_kernel_spmd  # noqa: E402

F32 = mybir.dt.float32
BF16 = mybir.dt.bfloat16
FP16 = mybir.dt.float16
FP8 = mybir.dt.float8e4
I16 = mybir.dt.int16
ALU = mybir.AluOpType
ACTF = mybir.ActivationFunctionType
BF = ml_dtypes.bfloat16

NUM_ITEM = 2000
DK = 256          # key dim
DV = 128          # memory slots (v)
B, T = 256, 512
NC = 8
BL = B // NC      # 32 local batches
KSUB = 4          # k quarters on partitions
K2 = DK // KSUB   # 64
P = BL * KSUB     # 128 partitions: p = b*4 + ksub
SEG = K2 * DV     # 8192 cells per partition (k2, v)
C = 4             # scan chunk length (time steps per scan instruction)
SLOT = C + 1      # per-cell slots in D/U (C data + 1 boundary)
NCH = T // C      # 128 chunks
NIT = 2048        # padded item count (16 tiles of 128)
NX = 4096         # padded x count (32 tiles of 128)
NXQ = NX * KSUB   # WEA table rows
TOK = BL * T      # 16384 tokens per core
NW = C * P // 16  # index columns per chunk (32)

FP8SCALE = 128.0  # k/v embeddings stored fp8 * FP8SCALE; 1/FP8SCALE folded into weights

# ---- shared-blob layout (2-byte elements, offsets in elements) ----
OFF_KT8 = 0                          # [DK, NIT] fp8 bits (x*128)
OFF_VT8 = OFF_KT8 + DK * NIT // 2    # [DK, NX] fp8 bits (x*128)
OFF_EAWT = OFF_VT8 + DK * NX // 2    # [DK, 2*DK] bf16 (/128)
OFF_MKT = OFF_EAWT + DK * 2 * DK     # [DK, DV] bf16 (/128)
OFF_FW2T = OFF_MKT + DK * DV         # [DK, DK] bf16 (/128)
OFF_FW1T = OFF_FW2T + DK * DK        # [DK, DK] fp16 bits
OFF_PWREP = OFF_FW1T + DK * DK       # [128, DK] fp16 bits
OFF_M0SH4 = OFF_PWREP + 128 * DK     # [4, SEG] fp16 bits (shifted, row=ks)
OFF_M0C0 = OFF_M0SH4 + 4 * SEG       # [P, 1] fp16 bits (pad to 2048)
OFF_F32S = OFF_M0C0 + 2048           # f32 smalls as u16 pairs (4096 u16)
#   f32 element offsets within the F32S region:
F_ONES = 0      # [1, 128]
F_EAB = 128     # [1, 2*DK]
F_FB = 640      # [1, DK]
F_PB = 896      # [128, 1]
OFF_KSV = OFF_F32S + 4096            # [128, 1] i16 (p % 4), pad to 2048
TOTELEM = OFF_KSV + 2048             # 1155072 = 8 * 144384
SHARD = TOTELEM // NC

# ---- per-core "allin" layout (2-byte elements) ----
AI_XW = SHARD                        # [4, NCH*NW] i16: 4*x at (blo, ch*32+tl*8+bhi)
AI_KFIDX = AI_XW + 4 * NCH * NW      # [16, TOK//16] i16 bits
AI_TOT = AI_KFIDX + 16 * (TOK // 16)

_cache = {}


def build_program():
    nc = bacc.Bacc(None, target_bir_lowering=False, debug=False)

    allin = nc.dram_tensor("allin", [AI_TOT], BF16, kind="ExternalInput")
    pred = nc.dram_tensor("pred", [128, TOK // 128], FP16, kind="ExternalOutput")

    # ---- DRAM scratch ----
    binb = nc.dram_tensor("binb", [SHARD], BF16)         # collective in bounce
    blob = nc.dram_tensor("blob", [TOTELEM], BF16, addr_space="Shared")
    WEAtab = nc.dram_tensor("WEAtab", [NXQ, 2 * DV], FP16)   # [w | e_q | a_q] rows
    KFtab = nc.dram_tensor("KFtab", [NIT, DK], FP16)
    rT_d = nc.dram_tensor("rT_d", [P, K2, NCH, C], FP16)     # reads, scan-native layout

    WEA4 = WEAtab[:].rearrange("(x q) c -> x q c", q=KSUB)   # [NX, 4, 256]

    def bview(off, p, n, dt=BF16):
        ap = blob[off:off + p * n]
        if dt is not BF16:
            ap = ap.bitcast(dt)
        return ap.rearrange("(p n) -> p n", n=n)

    def b8view(off, p, n):
        """fp8 view: off in u16 elems, p*n fp8 elems (= p*n/2 u16)."""
        return blob[off:off + p * n // 2].bitcast(FP8) \
            .rearrange("(p n) -> p n", n=n)

    def fview(foff, p, n):
        ap = blob[OFF_F32S + 2 * foff:OFF_F32S + 2 * (foff + p * n)].bitcast(F32)
        return ap.rearrange("(p n) -> p n", n=n)

    xw_v = allin[AI_XW:AI_XW + 4 * NCH * NW].bitcast(I16) \
        .rearrange("(p n) -> p n", n=NCH * NW)
    kfidx_v = allin[AI_KFIDX:AI_KFIDX + 16 * (TOK // 16)].bitcast(I16) \
        .rearrange("(p n) -> p n", n=TOK // 16)
    ksv_v = blob[OFF_KSV:OFF_KSV + P].bitcast(I16).rearrange("(p n) -> p n", n=1)

    with TileContext(nc) as tc:
        # ============ stage 0: AllGather the shared blob ============
        nc.gpsimd.dma_start(binb[:], allin[0:SHARD])
        nc.gpsimd.collective_compute(
            "AllGather", mybir.AluOpType.bypass,
            replica_groups=[list(range(NC))],
            ins=[binb[:].opt()], outs=[blob[:].opt()])

        # ================= stage 1+2: tables =================
        with (
            tc.tile_pool(name="wpool", bufs=1) as wp,
            tc.tile_pool(name="tpool", bufs=1) as tp,
            tc.tile_pool(name="pspool", bufs=2, space="PSUM") as pp,
        ):
            kT_s = [wp.tile([128, NIT], BF16, tag=f"kt{i}", name=f"kt{i}") for i in range(2)]
            vT_s = [wp.tile([128, NX], BF16, tag=f"vt{i}", name=f"vt{i}") for i in range(2)]
            kT8_s = [wp.tile([128, NIT], FP8, tag=f"kt8{i}", name=f"kt8{i}") for i in range(2)]
            vT8_s = [wp.tile([128, NX], FP8, tag=f"vt8{i}", name=f"vt8{i}") for i in range(2)]
            MkT_s = [wp.tile([128, DV], BF16, tag=f"mk{i}", name=f"mk{i}") for i in range(2)]
            eaWT_s = [wp.tile([128, 2 * DK], BF16, tag=f"ea{i}", name=f"eaw{i}") for i in range(2)]
            fW2T_s = [wp.tile([128, DK], BF16, tag=f"f2{i}", name=f"f2{i}") for i in range(2)]
            onesf_s = wp.tile([1, 128], F32, tag="onf")
            eab_s = wp.tile([1, 2 * DK], F32, tag="eb")
            fb_s = wp.tile([1, DK], F32, tag="fb")
            for i in range(2):
                nc.sync.dma_start(kT8_s[i][:], b8view(OFF_KT8 + 128 * i * NIT // 2, 128, NIT))
                nc.sync.dma_start(vT8_s[i][:], b8view(OFF_VT8 + 128 * i * NX // 2, 128, NX))
                nc.vector.tensor_copy(out=kT_s[i][:], in_=kT8_s[i][:])
                nc.vector.tensor_copy(out=vT_s[i][:], in_=vT8_s[i][:])
                nc.sync.dma_start(MkT_s[i][:], bview(OFF_MKT + 128 * i * DV, 128, DV))
                nc.sync.dma_start(eaWT_s[i][:],
                                  bview(OFF_EAWT + 128 * i * 2 * DK, 128, 2 * DK))
                nc.sync.dma_start(fW2T_s[i][:], bview(OFF_FW2T + 128 * i * DK, 128, DK))
            nc.sync.dma_start(onesf_s[:], fview(F_ONES, 1, 128))
            nc.sync.dma_start(eab_s[:], fview(F_EAB, 1, 2 * DK))
            nc.sync.dma_start(fb_s[:], fview(F_FB, 1, DK))

            # --- w part: softmax(k_emb @ Mk^T), fp16 ---
            wexp = tp.tile([128, 16, DV], F32, tag="wexp")
            for it in range(16):
                ps = pp.tile([128, DV], F32, tag="ps_w")
                sl = slice(128 * it, 128 * (it + 1))
                nc.tensor.matmul(out=ps[:], lhsT=kT_s[0][:, sl], rhs=MkT_s[0][:],
                                 start=True, stop=False)
                nc.tensor.matmul(out=ps[:], lhsT=kT_s[1][:, sl], rhs=MkT_s[1][:],
                                 start=False, stop=True)
                nc.scalar.activation(out=wexp[:, it, :], in_=ps[:], func=ACTF.Exp)
            zs = tp.tile([128, 16], F32, tag="zs")
            nc.vector.tensor_reduce(out=zs[:], in_=wexp[:], axis=mybir.AxisListType.X,
                                    op=ALU.add)
            zr = tp.tile([128, 16], F32, tag="zr")
            nc.vector.reciprocal(out=zr[:], in_=zs[:])
            wf16 = tp.tile([128, 16, DV], FP16, tag="wf16")
            nc.vector.tensor_tensor(
                out=wf16[:], in0=wexp[:],
                in1=zr[:].unsqueeze(2).to_broadcast([128, 16, DV]), op=ALU.mult)
            # WEA w columns: row (x*4+ks), x = 2000*corr + it*128 + p
            for corr in range(2):
                for ks in range(KSUB):
                    dst = WEA4[2000 * corr:2000 * corr + NIT, ks, 0:DV] \
                        .rearrange("(it p) c -> p it c", p=128)
                    nc.sync.dma_start(dst, wf16[:])

            # --- e/a parts: sigmoid/tanh(v_emb @ [eW|aW]^T + [eb|ab]), fp16 ---
            ea = tp.tile([128, 32, 2 * DK], FP16, tag="ea")
            for it in range(32):
                ps = pp.tile([128, 2 * DK], F32, tag="ps_ea")
                sl = slice(128 * it, 128 * (it + 1))
                nc.tensor.matmul(out=ps[:], lhsT=vT_s[0][:, sl], rhs=eaWT_s[0][:],
                                 start=True, stop=False)
                nc.tensor.matmul(out=ps[:], lhsT=vT_s[1][:, sl], rhs=eaWT_s[1][:],
                                 start=False, stop=False)
                nc.tensor.matmul(out=ps[:], lhsT=onesf_s[:], rhs=eab_s[:],
                                 start=False, stop=True)
                nc.scalar.activation(out=ea[:, it, 0:DK], in_=ps[:, 0:DK], func=ACTF.Sigmoid)
                nc.scalar.activation(out=ea[:, it, DK:2 * DK], in_=ps[:, DK:2 * DK],
                                     func=ACTF.Tanh)
            for q in range(KSUB):
                nc.sync.dma_start(
                    WEA4[:, q, DV:DV + K2].rearrange("(it p) c -> p it c", p=128),
                    ea[:, :, 64 * q:64 * (q + 1)])
                nc.sync.dma_start(
                    WEA4[:, q, DV + K2:2 * DV].rearrange("(it p) c -> p it c", p=128),
                    ea[:, :, DK + 64 * q:DK + 64 * (q + 1)])

            # --- KFtab: k_emb @ fW2^T + f_b, fp16 ---
            kf = tp.tile([128, 16, DK], FP16, tag="kf")
            for it in range(16):
                ps = pp.tile([128, DK], F32, tag="ps_kf")
                sl = slice(128 * it, 128 * (it + 1))
                nc.tensor.matmul(out=ps[:], lhsT=kT_s[0][:, sl], rhs=fW2T_s[0][:],
                                 start=True, stop=False)
                nc.tensor.matmul(out=ps[:], lhsT=kT_s[1][:, sl], rhs=fW2T_s[1][:],
                                 start=False, stop=False)
                nc.tensor.matmul(out=ps[:], lhsT=onesf_s[:], rhs=fb_s[:],
                                 start=False, stop=True)
                nc.scalar.copy(out=kf[:, it, :], in_=ps[:])
            nc.sync.dma_start(
                KFtab[:].rearrange("(it p) c -> p it c", p=128), kf[:])

        # ================= stage 3: the recurrence =================
        # The HW tensor_tensor_scan runs at ~0.4 elem/cycle (103 us per
        # 40960-elem chunk) while plain TTs hit ~3.7 elem/cycle, so the
        # time recurrence is unrolled as per-step multiply-adds instead:
        #   D_t = 1 - w_t e_t ; U_t = w_t a_t   (batched per chunk)
        #   U_t <- M_(t-1) * D_t + U_t          (U becomes the state M_t)
        #   read_t = sum_v w_t * M_(t-1)        (batched, products in D)
        with (
            tc.tile_pool(name="scst", bufs=1) as st,
            tc.tile_pool(name="scg", bufs=2) as sg,
            tc.tile_pool(name="scr", bufs=2) as rp,
        ):
            # cidx_s[p=16g+4blo+ks, :] = 4*x (row blo of xw) + ks
            cidx_s = st.tile([128, NCH * NW], I16, tag="cix")
            for g in range(8):
                for blo in range(4):
                    nc.sync.dma_start(
                        cidx_s[16 * g + 4 * blo:16 * g + 4 * blo + 4, :],
                        xw_v[blo:blo + 1, :].to_broadcast([4, NCH * NW]))
            ksv_s = st.tile([128, 1], I16, tag="ksv")
            nc.sync.dma_start(ksv_s[:], ksv_v)
            nc.vector.tensor_tensor(out=cidx_s[:], in0=cidx_s[:],
                                    in1=ksv_s[:].to_broadcast([128, NCH * NW]),
                                    op=ALU.add)

            # states S holds M_(-1)..M_(C-1): slot 0 = carry-in, build loads
            # slot t+1 with w_t*a_t and the update folds in the decayed state.
            Dt = st.tile([P, C * SEG], FP16, tag="D")
            St = st.tile([P, (C + 1) * SEG], FP16, tag="S")
            d4 = Dt[:].rearrange("p (t k v) -> p t k v", t=C, k=K2)
            s5 = St[:].rearrange("p (t k v) -> p t k v", t=C + 1, k=K2)
            # carry init: M0 rows depend only on ks = p%4
            m04 = bview(OFF_M0SH4, 4, SEG, FP16)
            for g in range(32):
                nc.sync.dma_start(St[4 * g:4 * (g + 1), 0:SEG], m04)

            def prefetch(ch):
                """gather [w | e_q | a_q] rows for chunk ch (step-major)."""
                wea_g = sg.tile([P, C, 2 * DV], FP16, tag="wg")
                nc.gpsimd.dma_gather(wea_g[:], WEAtab[:],
                                     cidx_s[:, ch * NW:(ch + 1) * NW],
                                     C * P, C * P, 2 * DV)
                return wea_g

            def build(wea_g):
                """D = 1 - w*e; S[1:] = w*a for all C steps of a chunk."""
                w_b = wea_g[:, :, 0:DV].unsqueeze(2).to_broadcast([P, C, K2, DV])
                e_b = wea_g[:, :, DV:DV + K2].unsqueeze(3) \
                    .to_broadcast([P, C, K2, DV])
                a_b = wea_g[:, :, DV + K2:2 * DV].unsqueeze(3) \
                    .to_broadcast([P, C, K2, DV])
                nc.vector.tensor_tensor(out=d4[:], in0=w_b, in1=e_b, op=ALU.mult)
                nc.vector.tensor_scalar(
                    out=d4[:], in0=d4[:],
                    scalar1=-1.0, scalar2=1.0, op0=ALU.mult, op1=ALU.add)
                nc.vector.tensor_tensor(out=s5[:, 1:C + 1], in0=w_b, in1=a_b,
                                        op=ALU.mult)

            wea_g_cur = prefetch(0)
            build(wea_g_cur)

            for ch in range(NCH):
                if ch + 1 < NCH:
                    wea_g_next = prefetch(ch + 1)

                # time steps: S[t+1] <- S[t]*D_t + S[t+1]  (D_t as scratch)
                for t in range(C):
                    nc.vector.tensor_tensor(out=d4[:, t], in0=d4[:, t],
                                            in1=s5[:, t], op=ALU.mult)
                    nc.vector.tensor_tensor(out=s5[:, t + 1], in0=s5[:, t + 1],
                                            in1=d4[:, t], op=ALU.add)

                # reads: one batched product M_(t-1)*w_t, then a pairwise
                # halving tree over v (7x faster than one innermost reduce)
                rT_sb = rp.tile([P, C, K2], F32, tag="rt")
                nc.vector.tensor_tensor(
                    out=d4[:], in0=s5[:, 0:C],
                    in1=wea_g_cur[:, :, 0:DV].unsqueeze(2)
                        .to_broadcast([P, C, K2, DV]), op=ALU.mult)
                for h in (64, 32, 16, 8, 4):
                    nc.vector.tensor_tensor(
                        out=d4[:, :, :, 0:h], in0=d4[:, :, :, 0:h],
                        in1=d4[:, :, :, h:2 * h], op=ALU.add)
                nc.vector.tensor_reduce(
                    out=rT_sb[:], in_=d4[:, :, :, 0:4],
                    axis=mybir.AxisListType.X, op=ALU.add)

                if ch + 1 < NCH:
                    # carry chunk-final state into slot 0, then build next
                    nc.scalar.copy(out=St[:, 0:SEG], in_=St[:, C * SEG:])
                    build(wea_g_next)

                rT_h = rp.tile([P, K2, C], FP16, tag="rth")
                nc.scalar.copy(out=rT_h[:], in_=rT_sb[:].transpose([0, 2, 1]))
                nc.sync.dma_start(rT_d[:, :, ch, :], rT_h[:])

                if ch + 1 < NCH:
                    wea_g_cur = wea_g_next

        # ================= stage 4: head =================
        with (
            tc.tile_pool(name="hw", bufs=1) as hw,
            tc.tile_pool(name="hp", bufs=1) as hpool,
            tc.tile_pool(name="hps", bufs=4, space="PSUM") as hps,
        ):
            fW1_s = [hw.tile([128, DK], FP16, tag=f"f1{i}", name=f"f1{i}") for i in range(2)]
            for i in range(2):
                nc.sync.dma_start(fW1_s[i][:],
                                  bview(OFF_FW1T + 128 * i * DK, 128, DK, FP16))
            # G1 = fW1 @ read, emitted token-major straight from PE
            # (lhsT = reads tile, rhs = fW1T) and kept in SBUF — no DRAM
            # roundtrip, no 2-byte-run transpose DMA.
            g1all = hw.tile([128, TOK // 128, DK], FP16, tag="g1all")
            QT = TOK // 4  # 4096 tokens
            for q in range(4):
                rq = [hpool.tile([128, QT], FP16, tag=f"rq{h}", name=f"rq{h}") for h in range(2)]
                for h in range(2):
                    for j in range(2):
                        ks = 2 * h + j
                        src = rT_d[:].rearrange(
                            "(b ks) k ch t -> ks k b ch t", ks=KSUB)[
                            ks, :, q * 8:(q + 1) * 8, :, :]
                        nc.sync.dma_start(rq[h][64 * j:64 * (j + 1), :], src)
                for blk in range(QT // 128):
                    ps = hps.tile([128, DK], F32, tag="psh")
                    bsl = slice(128 * blk, 128 * (blk + 1))
                    nc.tensor.matmul(out=ps[:], lhsT=rq[0][:, bsl], rhs=fW1_s[0][:],
                                     start=True, stop=False)
                    nc.tensor.matmul(out=ps[:], lhsT=rq[1][:, bsl], rhs=fW1_s[1][:],
                                     start=False, stop=True)
                    nc.scalar.copy(out=g1all[:, 32 * q + blk, :], in_=ps[:])

            # f = tanh(G1 + KF), pred = sigmoid(p.f + pb)
            pW_s = hw.tile([128, DK], FP16, tag="pw")
            pb_s = hw.tile([128, 1], F32, tag="pb")
            kfi_s = hw.tile([P, TOK // 16], I16, tag="kfi")
            nc.sync.dma_start(pW_s[:], bview(OFF_PWREP, 128, DK, FP16))
            nc.sync.dma_start(pb_s[:], fview(F_PB, 128, 1))
            for g in range(8):
                nc.sync.dma_start(kfi_s[16 * g:16 * (g + 1), :], kfidx_v)
            prow = hw.tile([128, TOK // 128], F32, tag="prow")
            for q in range(4):
                # tokens tok = q*4096 + blk*128 + p, blk in [0,32)
                g1q = g1all[:, 32 * q:32 * (q + 1), :]
                kfg = hpool.tile([128, 32, DK], FP16, tag="kfg")
                for g in range(4):
                    nc.gpsimd.dma_gather(
                        kfg[:, 8 * g:8 * (g + 1), :], KFtab[:],
                        kfi_s[:, (q * 4 + g) * 64:(q * 4 + g + 1) * 64],
                        1024, 1024, DK)
                fq = hpool.tile([128, 32, DK], FP16, tag="fq")
                nc.vector.tensor_tensor(out=fq[:], in0=g1q, in1=kfg[:], op=ALU.add)
                nc.scalar.activation(out=fq[:], in_=fq[:], func=ACTF.Tanh)
                nc.vector.tensor_tensor(
                    out=fq[:], in0=fq[:],
                    in1=pW_s[:].unsqueeze(1).to_broadcast([128, 32, DK]), op=ALU.mult)
                nc.vector.tensor_reduce(out=prow[:, 32 * q:32 * (q + 1)], in_=fq[:],
                                        axis=mybir.AxisListType.X, op=ALU.add)
            nc.scalar.activation(out=prow[:], in_=prow[:], func=ACTF.Sigmoid,
                                 bias=pb_s[:])
            predh = hw.tile([128, TOK // 128], FP16, tag="predh")
            nc.scalar.copy(out=predh[:], in_=prow[:])
            nc.sync.dma_start(pred[:], predh[:])

    nc.finalize()
    return nc


def _host_prep(item_seq, correct_seq, k_emb, v_emb, Mk, Mv0, e_W, e_b, a_W, a_b,
               f_W, f_b, p_W, p_b):
    """Pack the core-invariant 2-byte blob; return it split in NC shards."""
    blob = np.zeros(TOTELEM, np.uint16)

    def put2(off, arr):
        a = np.ascontiguousarray(arr)
        blob[off:off + a.size] = a.view(np.uint16).ravel()

    def put8(off, arr):
        """fp8 array -> u16 container (2 fp8 per u16)."""
        a = np.ascontiguousarray(arr)
        blob[off:off + a.size // 2] = a.view(np.uint8).ravel().view(np.uint16)

    pad_k = np.zeros((NIT, DK), np.float32)
    pad_k[:NUM_ITEM] = k_emb
    pad_v = np.zeros((NX, DK), np.float32)
    pad_v[:2 * NUM_ITEM] = v_emb
    put8(OFF_KT8, (pad_k.T * FP8SCALE).astype(ml_dtypes.float8_e4m3fn))
    put8(OFF_VT8, (pad_v.T * FP8SCALE).astype(ml_dtypes.float8_e4m3fn))
    s = np.float32(1.0 / FP8SCALE)
    put2(OFF_EAWT, (np.concatenate([e_W.T, a_W.T], axis=1) * s).astype(BF))
    put2(OFF_MKT, (Mk.T * s).astype(BF))
    put2(OFF_FW2T, (f_W[:, DK:].T * s).astype(BF))
    put2(OFF_FW1T, f_W[:, :DK].T.astype(np.float16))
    put2(OFF_PWREP, np.tile(p_W.reshape(1, DK), (128, 1)).astype(np.float16))
    put2(OFF_KSV, (np.arange(P) % KSUB).astype(np.int16).reshape(P, 1))

    # M0 cell layout depends only on ks = p % KSUB:
    # m0row(ks) = Mv0.T[ks*K2 + k2, v] flattened over (k2, v)
    k2i, vi = np.meshgrid(np.arange(K2), np.arange(DV), indexing="ij")
    ks4 = np.arange(KSUB)
    m04 = Mv0.T[(ks4[:, None, None] * K2 + k2i[None]), vi[None]].reshape(KSUB, SEG)
    put2(OFF_M0SH4, m04.astype(np.float16))

    f32s = np.zeros(2048, np.float32)
    f32s[F_ONES:F_ONES + 128] = 1.0
    f32s[F_EAB:F_EAB + 2 * DK] = np.concatenate([e_b, a_b])
    f32s[F_FB:F_FB + DK] = f_b
    f32s[F_PB:F_PB + 128] = float(p_b[0])
    put2(OFF_F32S, f32s.view(np.uint16))

    return blob.reshape(NC, SHARD)


def _wrap16(vals):
    """int array [n] -> [16, n/16] wrapped-in-16 (i%16, i//16)."""
    n = len(vals)
    return np.ascontiguousarray(
        np.asarray(vals, np.int64).reshape(n // 16, 16).T).astype(np.int16)


def _core_allin(blob_shard, item_c, x_c):
    """Per-core single 2-byte input: [blob shard | xw4m | kfidx16]."""
    allin = np.zeros(AI_TOT, np.uint16)
    allin[0:SHARD] = blob_shard
    # xw4m[blo, 32*ch+8*tl+bhi] = 4*x_c[4*bhi+blo, 4*ch+tl]
    A = x_c.reshape(P // 16, 4, NCH, C)               # [bhi, blo, ch, tl]
    xw = (4 * A).transpose(1, 2, 3, 0).reshape(4, NCH * NW)
    allin[AI_XW:AI_XW + xw.size] = \
        np.ascontiguousarray(xw.astype(np.int16)).view(np.uint16).ravel()
    kfidx = _wrap16(item_c.reshape(-1))
    allin[AI_KFIDX:AI_KFIDX + kfidx.size] = kfidx.view(np.uint16).ravel()
    return allin.view(BF)


def kernel(**inputs):
    inputs = {k: np.asarray(v) for k, v in inputs.items()}
    item = inputs["item_seq"].astype(np.int64)
    corr = inputs["correct_seq"].astype(np.int64)
    x = item + NUM_ITEM * corr

    if "nc" not in _cache:
        _cache["nc"] = build_program()
    nc = _cache["nc"]

    blob_sh = _host_prep(
        item, corr,
        inputs["k_emb"].astype(np.float32), inputs["v_emb"].astype(np.float32),
        inputs["Mk"].astype(np.float32), inputs["Mv0"].astype(np.float32),
        inputs["e_W"].astype(np.float32), inputs["e_b"].astype(np.float32),
        inputs["a_W"].astype(np.float32), inputs["a_b"].astype(np.float32),
        inputs["f_W"].astype(np.float32), inputs["f_b"].astype(np.float32),
        inputs["p_W"].astype(np.float32), inputs["p_b"].astype(np.float32))

    in_maps = []
    for c in range(NC):
        sl = slice(c * BL, (c + 1) * BL)
        in_maps.append({"allin": _core_allin(blob_sh[c], item[sl], x[sl])})

    res = run_bass_kernel_spmd(nc, in_maps, core_ids=list(range(NC)))

    out = np.zeros((B, T), np.float32)
    blk = np.arange(TOK // 128)
    pp_, bb_ = np.meshgrid(np.arange(128), blk, indexing="ij")
    tok = bb_ * 128 + pp_          # token id at [p, blk]
    for c in range(NC):
        pr = res.results[c]["pred"].astype(np.float32)   # [128, TOK//128]
        b_l, t_l = tok // T, tok % T
        out[c * BL + b_l, t_l] = pr
    return out


if __name__ == "__main__":
    # smoke test vs numpy reference
    import time
    rng = np.random.default_rng(0)
    s = 0.05
    ins = {
        "item_seq": rng.integers(0, NUM_ITEM, (B, T)),
        "correct_seq": rng.integers(0, 2, (B, T)),
        "k_emb": (rng.standard_normal((NUM_ITEM, DK)) * s).astype(np.float32),
        "v_emb": (rng.standard_normal((2 * NUM_ITEM, DK)) * s).astype(np.float32),
        "Mk": (rng.standard_normal((DV, DK)) * s).astype(np.float32),
        "Mv0": (rng.standard_normal((DV, DK)) * s).astype(np.float32),
        "e_W": (rng.standard_normal((DK, DK)) * s).astype(np.float32),
        "e_b": np.zeros(DK, np.float32),
        "a_W": (rng.standard_normal((DK, DK)) * s).astype(np.float32),
        "a_b": np.zeros(DK, np.float32),
        "f_W": (rng.standard_normal((DK, 2 * DK)) * s).astype(np.float32),
        "f_b": np.zeros(DK, np.float32),
        "p_W": (rng.standard_normal((1, DK)) * s).astype(np.float32),
        "p_b": np.zeros(1, np.float32),
    }
    t0 = time.time()
    out = kernel(**ins)
    print("kernel wall:", time.time() - t0)

    # numpy reference
    k = ins["k_emb"][ins["item_seq"]]
    v = ins["v_emb"][ins["item_seq"] + NUM_ITEM * ins["correct_seq"]]
    logits = k @ ins["Mk"].T
    w = np.exp(logits - logits.max(-1, keepdims=True))
    w /= w.sum(-1, keepdims=True)
    e = 1 / (1 + np.exp(-(v @ ins["e_W"].T + ins["e_b"])))
    a = np.tanh(v @ ins["a_W"].T + ins["a_b"])
    M = np.broadcast_to(ins["Mv0"][None], (B, DV, DK)).copy()
    reads = np.zeros((B, T, DK), np.float32)
    for t in range(T):
        reads[:, t] = np.einsum("bv,bvk->bk", w[:, t], M)
        M = M * (1 - w[:, t][:, :, None] * e[:, t][:, None, :]) \
            + w[:, t][:, :, None] * a[:, t][:, None, :]
    f = np.tanh(np.concatenate([reads, k], -1) @ ins["f_W"].T + ins["f_b"])
    ref = 1 / (1 + np.exp(-(f @ ins["p_W"].T + ins["p_b"])))[:, :, 0]
    err = np.abs(out - ref)
    print("max abs err:", err.max(), " rel:", err.max() / np.abs(ref).max())
